# revision 10
# baseline (speedup 1.0000x reference)
"""Trainium2 Bass kernel for CachedMistralAttention prefill (B=1, S=2048, H=4096,
32 q heads / 8 kv heads GQA, rotate-half RoPE, causal SDPA).

Sharding: tensor-parallel over heads across 8 NeuronCores. Core c owns q heads
[4c, 4c+4) and kv head c (one GQA group), computes its partial output
projection attn @ wo[4c:4c+4], and the host sums the 8 partials.

Per-core dataflow (all matmuls bf16 with fp32 PSUM accumulation):
  phase A: qT/kT/vT = W.T @ hiddenT per s-block; RoPE applied on the fly
           (rotate-half via partition-offset DVE reads); v transposed to
           [s, d] via DMA-transpose for use as the PV stationary operand.
  phase B: per (head, 512-wide sq block): scoresT[sk, sq] = kT.T @ qT tiles,
           exp on ScalarE (scale=1/sqrt(d)) with causal masking (skip upper
           tiles, zero left strip + triangular mask on diagonal tiles),
           attn_unnormT[d, sq] = v.T @ expT and denom[1, sq] = ones.T @ expT
           accumulated on PSUM, reciprocal + partition_broadcast + DVE mult
           for the normalization.
  phase C: out[sq, m] = sum_h attnT_h.T @ wo_h accumulated in PSUM over
           heads, evacuated to SBUF and DMA'd out as fp32.

All DRAM inputs are pre-tiled on the host so each DMA reads per-partition
contiguous spans (few, large descriptors - DMA issue is the scarce resource).
"""

import math
from contextlib import ExitStack

import numpy as np
import ml_dtypes

import concourse.bacc as bacc
import concourse.mybir as mybir
import concourse.tile as tile
from concourse.bass_utils import run_bass_kernel_spmd

BF16 = mybir.dt.bfloat16
F32 = mybir.dt.float32
NPBF16 = ml_dtypes.bfloat16

S = 2048          # sequence length
H = 4096          # hidden size
D = 128           # head dim
NH = 4            # q heads per core (one GQA group)
NCORES = 8
KT = H // 128     # 32 contraction tiles for the projections
SB = 512          # phase A s-block width
NSB = S // SB     # 4
KG = 16           # kt-tiles per hidden-strip sub-DMA chunk
SQB = 512         # phase B sq-block width
NSQB = S // SQB   # 4
NSK = S // 128    # 16 sk tiles
INV_NORM = 1.0 / math.sqrt(D)
MAX_WAVELENGTH = 10000.0


def _build_program():
    nc = bacc.Bacc("TRN2", target_bir_lowering=False, debug=False,
                   num_devices=NCORES)

    # pre-tiled inputs: leading dim 128 = SBUF partition, free dims contiguous
    hid_d = nc.dram_tensor("hidP", [NSB, 128, KT * SB], BF16, kind="ExternalInput")
    wq_d = nc.dram_tensor("wqP", [128, KT * NH * D], BF16, kind="ExternalInput")
    wk_d = nc.dram_tensor("wkP", [128, KT * D], BF16, kind="ExternalInput")
    wv_d = nc.dram_tensor("wvP", [128, KT * D], BF16, kind="ExternalInput")
    wo_d = nc.dram_tensor("woP", [128, NH * H], BF16, kind="ExternalInput")
    cos_d = nc.dram_tensor("cosT", [D, S], F32, kind="ExternalInput")
    sinA_d = nc.dram_tensor("sinA", [D, S], F32, kind="ExternalInput")
    tri_d = nc.dram_tensor("trimask", [D, D], BF16, kind="ExternalInput")
    out_d = nc.dram_tensor("out", [S, H], F32, kind="ExternalOutput")

    with tile.TileContext(nc) as tc, ExitStack() as ctx:
        # pools
        wqo_p = ctx.enter_context(tc.tile_pool(name="wqo", bufs=1))
        const_p = ctx.enter_context(tc.tile_pool(name="const", bufs=1))
        qkv_p = ctx.enter_context(tc.tile_pool(name="qkv", bufs=1))
        ps_p = None  # created per phase

        wq_sb = wqo_p.tile([128, KT, NH * D], BF16, tag="wqo")
        cos_sb = const_p.tile([D, S], F32, tag="cos")
        sinA_sb = const_p.tile([D, S], F32, tag="sin")
        tri_sb = const_p.tile([D, D], BF16, tag="tri")
        ones_sb = const_p.tile([128, 1], BF16, tag="ones")
        nc.vector.memset(ones_sb, 1.0)

        # persistent activations
        qT_sb = qkv_p.tile([128, NH, S], BF16, tag="qT")    # [d, h, s]
        kT_sb = qkv_p.tile([128, S], BF16, tag="kT")        # [d, s]
        vT_sb = qkv_p.tile([128, S], BF16, tag="vT")        # [d, s]
        v_sb = qkv_p.tile([128, NSK, D], BF16, tag="v")     # [s%128, skt, d]
        attnT_sb = qkv_p.tile([128, NH, S], BF16, tag="attnT")

        wq_v = wq_sb.rearrange("p kt n -> p (kt n)")

        # ---------------- phase A: projections + RoPE -----------------
        pha = ExitStack()
        psa_p = pha.enter_context(tc.tile_pool(name="psa", bufs=8, space="PSUM"))
        hid_p = pha.enter_context(tc.tile_pool(name="hid", bufs=2))
        wkv_p = pha.enter_context(tc.tile_pool(name="wkv", bufs=1))
        rope_p = pha.enter_context(tc.tile_pool(name="rope", bufs=2))
        wk_sb = wkv_p.tile([128, KT, D], BF16, tag="wk")
        wv_sb = wkv_p.tile([128, KT, D], BF16, tag="wv")
        for sb in range(NSB):
            ssl = slice(sb * SB, (sb + 1) * SB)
            hid_sb = hid_p.tile([128, KT, SB], BF16, tag="hid")
            hid_v = hid_sb.rearrange("p kt s -> p (kt s)")
            # strip DMA in kt-group chunks (contiguous on both sides);
            # finer chunks on the first strip so the first matmul starts early
            bounds = [0, 2, 4, 8, 16, 24, KT] if sb == 0 else \
                     list(range(0, KT + 1, KG))
            for g0, g1 in zip(bounds[:-1], bounds[1:]):
                nc.sync.dma_start(
                    out=hid_v[:, g0 * SB:g1 * SB],
                    in_=hid_d[:][sb, :, g0 * SB:g1 * SB])
                if sb == 0:
                    gb = NH * D
                    nc.scalar.dma_start(
                        out=wq_v[:, g0 * gb:g1 * gb],
                        in_=wq_d[:][:, g0 * gb:g1 * gb])
            if sb == 0:
                nc.scalar.dma_start(out=wk_sb.rearrange("p kt n -> p (kt n)"),
                                    in_=wk_d[:])
                nc.scalar.dma_start(out=wv_sb.rearrange("p kt n -> p (kt n)"),
                                    in_=wv_d[:])
                nc.scalar.dma_start(out=cos_sb, in_=cos_d[:])
                nc.scalar.dma_start(out=sinA_sb, in_=sinA_d[:])
                nc.scalar.dma_start(out=tri_sb, in_=tri_d[:])
            for t in range(NH + 2):  # 0..3 q heads, 4 = k, 5 = v
                ps = psa_p.tile([128, SB], F32, tag="psa")
                for kt in range(KT):
                    if t < NH:
                        lhsT = wq_sb[:, kt, t * D:(t + 1) * D]
                    elif t == NH:
                        lhsT = wk_sb[:, kt, :]
                    else:
                        lhsT = wv_sb[:, kt, :]
                    nc.tensor.matmul(ps, lhsT, hid_sb[:, kt, :],
                                     start=(kt == 0), stop=(kt == KT - 1))
                if t <= NH:
                    # RoPE: x*cos + rot(x)*sin, rot = [-x2, x1] (partition halves)
                    t1 = rope_p.tile([128, SB], F32, tag="t1")
                    t2 = rope_p.tile([128, SB], F32, tag="t2")
                    nc.vector.tensor_mul(t1, ps, cos_sb[:, ssl])
                    nc.vector.tensor_mul(t2[0:64, :], ps[64:128, :],
                                         sinA_sb[0:64, ssl])
                    nc.vector.tensor_mul(t2[64:128, :], ps[0:64, :],
                                         sinA_sb[64:128, ssl])
                    dst = qT_sb[:, t, ssl] if t < NH else kT_sb[:, ssl]
                    nc.vector.tensor_add(dst, t1, t2)
                else:
                    # v: evacuate to bf16; transposed in one batch at end of
                    # phase A (minimizes DMA xbar-mode transitions)
                    nc.scalar.copy(vT_sb[:, ssl], ps)

        for skt in range(NSK):
            nc.sync.dma_start_transpose(
                out=v_sb[:, skt, :], in_=vT_sb[:, skt * 128:(skt + 1) * 128])
        pha.close()

        # ---------------- phase B: attention per head -----------------
        phb = ExitStack()
        ps_p = ctx.enter_context(tc.tile_pool(name="ps", bufs=5, space="PSUM"))
        psat_p = phb.enter_context(tc.tile_pool(name="psat", bufs=2, space="PSUM"))
        psden_p = phb.enter_context(tc.tile_pool(name="psden", bufs=1, space="PSUM"))
        exp_p = phb.enter_context(tc.tile_pool(name="expp", bufs=8))
        rec_p = phb.enter_context(tc.tile_pool(name="recp", bufs=2))
        for h in range(NH):
            for b in range(NSQB):
                qsl = slice(b * SQB, (b + 1) * SQB)
                nsk = (b + 1) * (SQB // 128)
                ps_at = psat_p.tile([128, SQB], F32, tag="at")
                ps_den = psden_p.tile([1, SQB], F32, tag="den")
                for skt in range(nsk):
                    # diagonal blocks (j >= 0): columns sq < skt*128 are fully
                    # causal-masked - skip them in QK/PV/den and zero them in e
                    j = skt - b * (SQB // 128)
                    lo = max(j, 0) * 128      # first live column in this block
                    ps_sc = ps_p.tile([128, SQB], F32, tag="ps")
                    nc.tensor.matmul(ps_sc[:, lo:],
                                     kT_sb[:, skt * 128:(skt + 1) * 128],
                                     qT_sb[:, h, b * SQB + lo:(b + 1) * SQB],
                                     start=True, stop=True)
                    e = exp_p.tile([128, SQB], BF16, tag="e")
                    nc.scalar.activation(e[:, lo:], ps_sc[:, lo:],
                                         mybir.ActivationFunctionType.Exp,
                                         scale=INV_NORM)
                    if j >= 0:
                        # triangular mask on the [128,128] diagonal tile
                        nc.vector.tensor_mul(e[:, lo:lo + 128],
                                             e[:, lo:lo + 128], tri_sb)
                    nc.tensor.matmul(ps_at[:, lo:], v_sb[:, skt, :], e[:, lo:],
                                     start=(skt == 0), stop=(skt == nsk - 1))
                    nc.tensor.matmul(ps_den[:, lo:], ones_sb, e[:, lo:],
                                     start=(skt == 0), stop=(skt == nsk - 1))
                rec = rec_p.tile([1, SQB], F32, tag="rec")
                nc.vector.reciprocal(rec, ps_den)
                recb = rec_p.tile([128, SQB], F32, tag="recb")
                nc.gpsimd.partition_broadcast(recb, rec)
                nc.vector.tensor_mul(attnT_sb[:, h, qsl], ps_at, recb)

        phb.close()

        # ---------------- phase C: output projection ------------------
        out_p = ctx.enter_context(tc.tile_pool(name="outp", bufs=2))
        wo_sb = wqo_p.tile([128, NH, H], BF16, tag="wqo")
        nc.scalar.dma_start(out=wo_sb.rearrange("p h m -> p (h m)"), in_=wo_d[:])
        NMB = H // SQB  # 8 column blocks of 512
        HB = NMB // 2   # 4 blocks per half-row
        for sqt in range(S // 128):
            for half in range(2):
                o_sb = out_p.tile([128, HB * SQB], F32, tag="o")
                pss = [ps_p.tile([128, SQB], F32, tag="ps",
                                 name=f"pso_{sqt}_{half}_{i}")
                       for i in range(HB)]
                for hh in range(NH):
                    lhsT = attnT_sb[:, hh, sqt * 128:(sqt + 1) * 128]
                    for i in range(HB):
                        mb = half * HB + i
                        nc.tensor.matmul(pss[i], lhsT,
                                         wo_sb[:, hh, mb * SQB:(mb + 1) * SQB],
                                         start=(hh == 0), stop=(hh == NH - 1))
                for i in range(HB):
                    if i % 2 == 0:
                        nc.scalar.copy(o_sb[:, i * SQB:(i + 1) * SQB], pss[i])
                    else:
                        nc.vector.tensor_copy(o_sb[:, i * SQB:(i + 1) * SQB],
                                              pss[i])
                nc.sync.dma_start(
                    out=out_d[:][sqt * 128:(sqt + 1) * 128,
                                 half * HB * SQB:(half + 1) * HB * SQB],
                    in_=o_sb)

    nc.compile()
    return nc


_NC = None


def _get_nc():
    global _NC
    if _NC is None:
        _NC = _build_program()
    return _NC


def _host_tables():
    pos = np.arange(S, dtype=np.float32)
    inv_freq = 1.0 / (MAX_WAVELENGTH ** (np.arange(0, D, 2, dtype=np.float32) / D))
    freq = np.einsum('i,j->ij', pos, inv_freq)          # [S, 64]
    emb = np.concatenate([freq, freq], axis=1)          # [S, 128]
    cosT = np.ascontiguousarray(np.cos(emb).T).astype(np.float32)   # [128, S]
    sinT = np.sin(emb).T.astype(np.float32)
    sinA = sinT.copy()
    sinA[:64] = -sinT[:64]
    sinA = np.ascontiguousarray(sinA)
    tri = np.triu(np.ones((D, D), dtype=np.float32)).astype(NPBF16)  # p<=f keep
    return cosT, sinA, tri


def _prepare_in_maps(hidden_states, wq, wk, wv, wo):
    hs = np.asarray(hidden_states, dtype=np.float32)[0]        # [S, H]
    wq = np.asarray(wq, dtype=np.float32)                      # [H, 32, 128]
    wk = np.asarray(wk, dtype=np.float32)                      # [H, 8, 128]
    wv = np.asarray(wv, dtype=np.float32)
    wo = np.asarray(wo, dtype=np.float32)                      # [32, 128, H]

    # hidP[sb, p, kt*SB + s] = hiddenT[kt*128 + p, sb*SB + s]
    hidT = hs.T.astype(NPBF16)                                 # [H, S]
    hidP = np.ascontiguousarray(
        hidT.reshape(KT, 128, NSB, SB).transpose(2, 1, 0, 3).reshape(
            NSB, 128, KT * SB))
    cosT, sinA, tri = _host_tables()

    def ptile(w2d):  # [H, N] -> [128, KT*N] with (p, kt*N+n) = w2d[kt*128+p, n]
        n = w2d.shape[1]
        return np.ascontiguousarray(
            w2d.reshape(KT, 128, n).transpose(1, 0, 2).reshape(128, KT * n))

    in_maps = []
    for c in range(NCORES):
        wq_c = wq[:, NH * c:NH * (c + 1), :].reshape(H, NH * D).astype(NPBF16)
        wk_c = wk[:, c, :].astype(NPBF16)
        wv_c = wv[:, c, :].astype(NPBF16)
        wo_c = wo[NH * c:NH * (c + 1)].reshape(NH * D, H).astype(NPBF16)
        woP = np.ascontiguousarray(
            wo_c.reshape(NH, 128, H).transpose(1, 0, 2).reshape(128, NH * H))
        in_maps.append({
            "hidP": hidP,
            "wqP": ptile(wq_c),
            "wkP": ptile(wk_c),
            "wvP": ptile(wv_c),
            "woP": woP,
            "cosT": cosT,
            "sinA": sinA,
            "trimask": tri,
        })
    return in_maps


def _run(in_maps, **kwargs):
    return run_bass_kernel_spmd(_get_nc(), in_maps,
                                core_ids=list(range(NCORES)), **kwargs)


def _gather(res):
    out = np.zeros((S, H), dtype=np.float32)
    for c in range(NCORES):
        out += np.asarray(res.results[c]["out"], dtype=np.float32)
    return out[None]


def kernel(hidden_states, attention_mask=None, wq=None, wk=None, wv=None, wo=None):
    in_maps = _prepare_in_maps(hidden_states, wq, wk, wv, wo)
    return _gather(_run(in_maps))


# revision 17
# speedup vs baseline: 1.0021x; 1.0021x over previous
"""Trainium2 Bass kernel for CachedMistralAttention prefill (B=1, S=2048, H=4096,
32 q heads / 8 kv heads GQA, rotate-half RoPE, causal SDPA).

Sharding: tensor-parallel over heads across 8 NeuronCores. Core c owns q heads
[4c, 4c+4) and kv head c (one GQA group), computes its partial output
projection attn @ wo[4c:4c+4], and the host sums the 8 partials.

Per-core dataflow (all matmuls bf16 with fp32 PSUM accumulation):
  phase A: qT/kT/vT = W.T @ hiddenT per s-block; RoPE applied on the fly
           (rotate-half via partition-offset DVE reads); v transposed to
           [s, d] via DMA-transpose for use as the PV stationary operand.
  phase B: per (head, 512-wide sq block): scoresT[sk, sq] = kT.T @ qT tiles,
           exp on ScalarE (scale=1/sqrt(d)) with causal masking (skip upper
           tiles, zero left strip + triangular mask on diagonal tiles),
           attn_unnormT[d, sq] = v.T @ expT and denom[1, sq] = ones.T @ expT
           accumulated on PSUM, reciprocal + partition_broadcast + DVE mult
           for the normalization.
  phase C: out[sq, m] = sum_h attnT_h.T @ wo_h accumulated in PSUM over
           heads, evacuated to SBUF and DMA'd out as fp32.

All DRAM inputs are pre-tiled on the host so each DMA reads per-partition
contiguous spans (few, large descriptors - DMA issue is the scarce resource).
"""

import math
from contextlib import ExitStack

import numpy as np
import ml_dtypes

import concourse.bacc as bacc
import concourse.mybir as mybir
import concourse.tile as tile
from concourse.bass_utils import run_bass_kernel_spmd

BF16 = mybir.dt.bfloat16
F32 = mybir.dt.float32
NPBF16 = ml_dtypes.bfloat16

S = 2048          # sequence length
H = 4096          # hidden size
D = 128           # head dim
NH = 4            # q heads per core (one GQA group)
NCORES = 8
KT = H // 128     # 32 contraction tiles for the projections
SB = 512          # phase A s-block width
NSB = S // SB     # 4
KG = 16           # kt-tiles per hidden-strip sub-DMA chunk
SQB = 512         # phase B sq-block width
NSQB = S // SQB   # 4
NSK = S // 128    # 16 sk tiles
INV_NORM = 1.0 / math.sqrt(D)
MAX_WAVELENGTH = 10000.0


def _build_program():
    nc = bacc.Bacc("TRN2", target_bir_lowering=False, debug=False,
                   num_devices=NCORES)

    # pre-tiled inputs: leading dim 128 = SBUF partition, free dims contiguous
    hid_d = nc.dram_tensor("hidP", [NSB, 128, KT * SB], BF16, kind="ExternalInput")
    wq_d = nc.dram_tensor("wqP", [128, KT * NH * D], BF16, kind="ExternalInput")
    wk_d = nc.dram_tensor("wkP", [128, KT * D], BF16, kind="ExternalInput")
    wv_d = nc.dram_tensor("wvP", [128, KT * D], BF16, kind="ExternalInput")
    wo_d = nc.dram_tensor("woP", [128, NH * H], BF16, kind="ExternalInput")
    cos_d = nc.dram_tensor("cosT", [D, S], F32, kind="ExternalInput")
    sinA_d = nc.dram_tensor("sinA", [D, S], F32, kind="ExternalInput")
    tri_d = nc.dram_tensor("trimask", [D, D], BF16, kind="ExternalInput")
    out_d = nc.dram_tensor("out", [S, H], F32, kind="ExternalOutput")

    with tile.TileContext(nc) as tc, ExitStack() as ctx:
        # pools
        wqo_p = ctx.enter_context(tc.tile_pool(name="wqo", bufs=1))
        const_p = ctx.enter_context(tc.tile_pool(name="const", bufs=1))
        qkv_p = ctx.enter_context(tc.tile_pool(name="qkv", bufs=1))
        ps_p = None  # created per phase

        wq_sb = wqo_p.tile([128, KT, NH * D], BF16, tag="wqo")
        cos_sb = const_p.tile([D, S], F32, tag="cos")
        sinA_sb = const_p.tile([D, S], F32, tag="sin")
        tri_sb = const_p.tile([D, D], BF16, tag="tri")
        ones_sb = const_p.tile([128, 1], BF16, tag="ones")
        nc.vector.memset(ones_sb, 1.0)

        # persistent activations
        qT_sb = qkv_p.tile([128, NH, S], BF16, tag="qT")    # [d, h, s]
        kT_sb = qkv_p.tile([128, S], BF16, tag="kT")        # [d, s]
        vT_sb = qkv_p.tile([128, S], BF16, tag="vT")        # [d, s]
        v_sb = qkv_p.tile([128, NSK, D], BF16, tag="v")     # [s%128, skt, d]
        attnT_sb = qkv_p.tile([128, NH, S], BF16, tag="attnT")

        wq_v = wq_sb.rearrange("p kt n -> p (kt n)")

        # ---------------- phase A: projections + RoPE -----------------
        pha = ExitStack()
        psa_p = pha.enter_context(tc.tile_pool(name="psa", bufs=8, space="PSUM"))
        hid_p = pha.enter_context(tc.tile_pool(name="hid", bufs=2))
        wkv_p = pha.enter_context(tc.tile_pool(name="wkv", bufs=1))
        rope_p = pha.enter_context(tc.tile_pool(name="rope", bufs=2))
        wk_sb = wkv_p.tile([128, KT, D], BF16, tag="wk")
        wv_sb = wkv_p.tile([128, KT, D], BF16, tag="wv")
        for sb in range(NSB):
            ssl = slice(sb * SB, (sb + 1) * SB)
            hid_sb = hid_p.tile([128, KT, SB], BF16, tag="hid")
            hid_v = hid_sb.rearrange("p kt s -> p (kt s)")
            # strip DMA in kt-group chunks (contiguous on both sides);
            # finer chunks on the first strip so the first matmul starts early
            bounds = [0, 2, 4, 8, 16, 24, KT] if sb == 0 else \
                     list(range(0, KT + 1, KG))
            for g0, g1 in zip(bounds[:-1], bounds[1:]):
                nc.sync.dma_start(
                    out=hid_v[:, g0 * SB:g1 * SB],
                    in_=hid_d[:][sb, :, g0 * SB:g1 * SB])
                if sb == 0:
                    gb = NH * D
                    nc.scalar.dma_start(
                        out=wq_v[:, g0 * gb:g1 * gb],
                        in_=wq_d[:][:, g0 * gb:g1 * gb])
            if sb == 0:
                nc.scalar.dma_start(out=wk_sb.rearrange("p kt n -> p (kt n)"),
                                    in_=wk_d[:])
                nc.scalar.dma_start(out=wv_sb.rearrange("p kt n -> p (kt n)"),
                                    in_=wv_d[:])
                nc.scalar.dma_start(out=cos_sb, in_=cos_d[:])
                nc.scalar.dma_start(out=sinA_sb, in_=sinA_d[:])
                nc.scalar.dma_start(out=tri_sb, in_=tri_d[:])
            for t in range(NH + 2):  # 0..3 q heads, 4 = k, 5 = v
                ps = psa_p.tile([128, SB], F32, tag="psa")
                for kt in range(KT):
                    if t < NH:
                        lhsT = wq_sb[:, kt, t * D:(t + 1) * D]
                    elif t == NH:
                        lhsT = wk_sb[:, kt, :]
                    else:
                        lhsT = wv_sb[:, kt, :]
                    nc.tensor.matmul(ps, lhsT, hid_sb[:, kt, :],
                                     start=(kt == 0), stop=(kt == KT - 1))
                if t <= NH:
                    # RoPE: x*cos + rot(x)*sin, rot = [-x2, x1] (partition halves)
                    t1 = rope_p.tile([128, SB], F32, tag="t1")
                    t2 = rope_p.tile([128, SB], F32, tag="t2")
                    nc.vector.tensor_mul(t1, ps, cos_sb[:, ssl])
                    nc.vector.tensor_mul(t2[0:64, :], ps[64:128, :],
                                         sinA_sb[0:64, ssl])
                    nc.vector.tensor_mul(t2[64:128, :], ps[0:64, :],
                                         sinA_sb[64:128, ssl])
                    dst = qT_sb[:, t, ssl] if t < NH else kT_sb[:, ssl]
                    nc.vector.tensor_add(dst, t1, t2)
                else:
                    # v: evacuate to bf16; transposed in one batch at end of
                    # phase A (minimizes DMA xbar-mode transitions)
                    nc.scalar.copy(vT_sb[:, ssl], ps)

        for skt in range(NSK):
            nc.sync.dma_start_transpose(
                out=v_sb[:, skt, :], in_=vT_sb[:, skt * 128:(skt + 1) * 128])
        pha.close()

        # ---------------- phase B: attention per head -----------------
        phb = ExitStack()
        ps_p = ctx.enter_context(tc.tile_pool(name="ps", bufs=5, space="PSUM"))
        psat_p = phb.enter_context(tc.tile_pool(name="psat", bufs=2, space="PSUM"))
        psden_p = phb.enter_context(tc.tile_pool(name="psden", bufs=1, space="PSUM"))
        exp_p = phb.enter_context(tc.tile_pool(name="expp", bufs=8))
        rec_p = phb.enter_context(tc.tile_pool(name="recp", bufs=2))
        for h in range(NH):
            for b in range(NSQB):
                qsl = slice(b * SQB, (b + 1) * SQB)
                nsk = (b + 1) * (SQB // 128)
                ps_at = psat_p.tile([128, SQB], F32, tag="at")
                ps_den = psden_p.tile([1, SQB], F32, tag="den")
                for skt in range(nsk):
                    # diagonal blocks (j >= 0): columns sq < skt*128 are fully
                    # causal-masked - skip them in QK/PV/den and zero them in e
                    j = skt - b * (SQB // 128)
                    lo = max(j, 0) * 128      # first live column in this block
                    ps_sc = ps_p.tile([128, SQB], F32, tag="ps")
                    nc.tensor.matmul(ps_sc[:, lo:],
                                     kT_sb[:, skt * 128:(skt + 1) * 128],
                                     qT_sb[:, h, b * SQB + lo:(b + 1) * SQB],
                                     start=True, stop=True)
                    e = exp_p.tile([128, SQB], BF16, tag="e")
                    nc.scalar.activation(e[:, lo:], ps_sc[:, lo:],
                                         mybir.ActivationFunctionType.Exp,
                                         scale=INV_NORM)
                    if j >= 0:
                        # triangular mask on the [128,128] diagonal tile
                        nc.vector.tensor_mul(e[:, lo:lo + 128],
                                             e[:, lo:lo + 128], tri_sb)
                    nc.tensor.matmul(ps_at[:, lo:], v_sb[:, skt, :], e[:, lo:],
                                     start=(skt == 0), stop=(skt == nsk - 1))
                    nc.tensor.matmul(ps_den[:, lo:], ones_sb, e[:, lo:],
                                     start=(skt == 0), stop=(skt == nsk - 1))
                rec = rec_p.tile([1, SQB], F32, tag="rec")
                nc.vector.reciprocal(rec, ps_den)
                recb = rec_p.tile([128, SQB], F32, tag="recb")
                nc.gpsimd.partition_broadcast(recb, rec)
                nc.vector.tensor_mul(attnT_sb[:, h, qsl], ps_at, recb)

        phb.close()

        # ---------------- phase C: output projection ------------------
        out_p = ctx.enter_context(tc.tile_pool(name="outp", bufs=2))
        wo_sb = wqo_p.tile([128, NH, H], BF16, tag="wqo")
        nc.scalar.dma_start(out=wo_sb.rearrange("p h m -> p (h m)"), in_=wo_d[:])
        NMB = H // SQB  # 8 column blocks of 512
        HB = NMB // 2   # 4 blocks per half-row
        for sqt in range(S // 128):
            for half in range(2):
                o_sb = out_p.tile([128, HB * SQB], F32, tag="o")
                pss = [ps_p.tile([128, SQB], F32, tag="ps",
                                 name=f"pso_{sqt}_{half}_{i}")
                       for i in range(HB)]
                for hh in range(NH):
                    lhsT = attnT_sb[:, hh, sqt * 128:(sqt + 1) * 128]
                    for i in range(HB):
                        mb = half * HB + i
                        nc.tensor.matmul(pss[i], lhsT,
                                         wo_sb[:, hh, mb * SQB:(mb + 1) * SQB],
                                         start=(hh == 0), stop=(hh == NH - 1))
                last = (sqt == S // 128 - 1)
                for i in range(HB):
                    if i % 2 == 0:
                        nc.scalar.copy(o_sb[:, i * SQB:(i + 1) * SQB], pss[i])
                    else:
                        nc.vector.tensor_copy(o_sb[:, i * SQB:(i + 1) * SQB],
                                              pss[i])
                    if last:
                        # fine-grained tail DMAs so the drain isn't gated on
                        # one big final transfer
                        mb = half * HB + i
                        nc.sync.dma_start(
                            out=out_d[:][sqt * 128:(sqt + 1) * 128,
                                         mb * SQB:(mb + 1) * SQB],
                            in_=o_sb[:, i * SQB:(i + 1) * SQB])
                if not last:
                    nc.sync.dma_start(
                        out=out_d[:][sqt * 128:(sqt + 1) * 128,
                                     half * HB * SQB:(half + 1) * HB * SQB],
                        in_=o_sb)

    nc.compile()
    return nc


_NC = None


def _get_nc():
    global _NC
    if _NC is None:
        _NC = _build_program()
    return _NC


def _host_tables():
    pos = np.arange(S, dtype=np.float32)
    inv_freq = 1.0 / (MAX_WAVELENGTH ** (np.arange(0, D, 2, dtype=np.float32) / D))
    freq = np.einsum('i,j->ij', pos, inv_freq)          # [S, 64]
    emb = np.concatenate([freq, freq], axis=1)          # [S, 128]
    cosT = np.ascontiguousarray(np.cos(emb).T).astype(np.float32)   # [128, S]
    sinT = np.sin(emb).T.astype(np.float32)
    sinA = sinT.copy()
    sinA[:64] = -sinT[:64]
    sinA = np.ascontiguousarray(sinA)
    tri = np.triu(np.ones((D, D), dtype=np.float32)).astype(NPBF16)  # p<=f keep
    return cosT, sinA, tri


def _prepare_in_maps(hidden_states, wq, wk, wv, wo):
    hs = np.asarray(hidden_states, dtype=np.float32)[0]        # [S, H]
    wq = np.asarray(wq, dtype=np.float32)                      # [H, 32, 128]
    wk = np.asarray(wk, dtype=np.float32)                      # [H, 8, 128]
    wv = np.asarray(wv, dtype=np.float32)
    wo = np.asarray(wo, dtype=np.float32)                      # [32, 128, H]

    # hidP[sb, p, kt*SB + s] = hiddenT[kt*128 + p, sb*SB + s]
    hidT = hs.T.astype(NPBF16)                                 # [H, S]
    hidP = np.ascontiguousarray(
        hidT.reshape(KT, 128, NSB, SB).transpose(2, 1, 0, 3).reshape(
            NSB, 128, KT * SB))
    cosT, sinA, tri = _host_tables()

    def ptile(w2d):  # [H, N] -> [128, KT*N] with (p, kt*N+n) = w2d[kt*128+p, n]
        n = w2d.shape[1]
        return np.ascontiguousarray(
            w2d.reshape(KT, 128, n).transpose(1, 0, 2).reshape(128, KT * n))

    in_maps = []
    for c in range(NCORES):
        wq_c = wq[:, NH * c:NH * (c + 1), :].reshape(H, NH * D).astype(NPBF16)
        wk_c = wk[:, c, :].astype(NPBF16)
        wv_c = wv[:, c, :].astype(NPBF16)
        wo_c = wo[NH * c:NH * (c + 1)].reshape(NH * D, H).astype(NPBF16)
        woP = np.ascontiguousarray(
            wo_c.reshape(NH, 128, H).transpose(1, 0, 2).reshape(128, NH * H))
        in_maps.append({
            "hidP": hidP,
            "wqP": ptile(wq_c),
            "wkP": ptile(wk_c),
            "wvP": ptile(wv_c),
            "woP": woP,
            "cosT": cosT,
            "sinA": sinA,
            "trimask": tri,
        })
    return in_maps


def _run(in_maps, **kwargs):
    return run_bass_kernel_spmd(_get_nc(), in_maps,
                                core_ids=list(range(NCORES)), **kwargs)


def _gather(res):
    out = np.zeros((S, H), dtype=np.float32)
    for c in range(NCORES):
        out += np.asarray(res.results[c]["out"], dtype=np.float32)
    return out[None]


def kernel(hidden_states, attention_mask=None, wq=None, wk=None, wv=None, wo=None):
    in_maps = _prepare_in_maps(hidden_states, wq, wk, wv, wo)
    return _gather(_run(in_maps))


# revision 18
# speedup vs baseline: 1.0024x; 1.0003x over previous
"""Trainium2 Bass kernel for CachedMistralAttention prefill (B=1, S=2048, H=4096,
32 q heads / 8 kv heads GQA, rotate-half RoPE, causal SDPA).

Sharding: tensor-parallel over heads across 8 NeuronCores. Core c owns q heads
[4c, 4c+4) and kv head c (one GQA group), computes its partial output
projection attn @ wo[4c:4c+4], and the host sums the 8 partials.

Per-core dataflow (all matmuls bf16 with fp32 PSUM accumulation):
  phase A: qT/kT/vT = W.T @ hiddenT per s-block; RoPE applied on the fly
           (rotate-half via partition-offset DVE reads); v transposed to
           [s, d] via DMA-transpose for use as the PV stationary operand.
  phase B: per (head, 512-wide sq block): scoresT[sk, sq] = kT.T @ qT tiles,
           exp on ScalarE (scale=1/sqrt(d)) with causal masking (skip upper
           tiles, zero left strip + triangular mask on diagonal tiles),
           attn_unnormT[d, sq] = v.T @ expT and denom[1, sq] = ones.T @ expT
           accumulated on PSUM, reciprocal + partition_broadcast + DVE mult
           for the normalization.
  phase C: out[sq, m] = sum_h attnT_h.T @ wo_h accumulated in PSUM over
           heads, evacuated to SBUF and DMA'd out as fp32.

All DRAM inputs are pre-tiled on the host so each DMA reads per-partition
contiguous spans (few, large descriptors - DMA issue is the scarce resource).
"""

import math
from contextlib import ExitStack

import numpy as np
import ml_dtypes

import concourse.bacc as bacc
import concourse.mybir as mybir
import concourse.tile as tile
from concourse.bass_utils import run_bass_kernel_spmd

BF16 = mybir.dt.bfloat16
F32 = mybir.dt.float32
NPBF16 = ml_dtypes.bfloat16

S = 2048          # sequence length
H = 4096          # hidden size
D = 128           # head dim
NH = 4            # q heads per core (one GQA group)
NCORES = 8
KT = H // 128     # 32 contraction tiles for the projections
SB = 512          # phase A s-block width
NSB = S // SB     # 4
KG = 16           # kt-tiles per hidden-strip sub-DMA chunk
SQB = 512         # phase B sq-block width
NSQB = S // SQB   # 4
NSK = S // 128    # 16 sk tiles
INV_NORM = 1.0 / math.sqrt(D)
MAX_WAVELENGTH = 10000.0


def _build_program():
    nc = bacc.Bacc("TRN2", target_bir_lowering=False, debug=False,
                   num_devices=NCORES)

    # pre-tiled inputs: leading dim 128 = SBUF partition, free dims contiguous
    hid_d = nc.dram_tensor("hidP", [NSB, 128, KT * SB], BF16, kind="ExternalInput")
    wq_d = nc.dram_tensor("wqP", [128, KT * NH * D], BF16, kind="ExternalInput")
    wk_d = nc.dram_tensor("wkP", [128, KT * D], BF16, kind="ExternalInput")
    wv_d = nc.dram_tensor("wvP", [128, KT * D], BF16, kind="ExternalInput")
    wo_d = nc.dram_tensor("woP", [128, NH * H], BF16, kind="ExternalInput")
    cos_d = nc.dram_tensor("cosT", [D, S], F32, kind="ExternalInput")
    sinA_d = nc.dram_tensor("sinA", [D, S], F32, kind="ExternalInput")
    tri_d = nc.dram_tensor("trimask", [D, D], BF16, kind="ExternalInput")
    out_d = nc.dram_tensor("out", [S, H], F32, kind="ExternalOutput")

    with tile.TileContext(nc) as tc, ExitStack() as ctx:
        # pools
        wqo_p = ctx.enter_context(tc.tile_pool(name="wqo", bufs=1))
        const_p = ctx.enter_context(tc.tile_pool(name="const", bufs=1))
        qkv_p = ctx.enter_context(tc.tile_pool(name="qkv", bufs=1))
        ps_p = None  # created per phase

        wq_sb = wqo_p.tile([128, KT, NH * D], BF16, tag="wqo")
        cos_sb = const_p.tile([D, S], F32, tag="cos")
        sinA_sb = const_p.tile([D, S], F32, tag="sin")
        tri_sb = const_p.tile([D, D], BF16, tag="tri")
        ones_sb = const_p.tile([128, 1], BF16, tag="ones")
        nc.vector.memset(ones_sb, 1.0)

        # persistent activations
        qT_sb = qkv_p.tile([128, NH, S], BF16, tag="qT")    # [d, h, s]
        kT_sb = qkv_p.tile([128, S], BF16, tag="kT")        # [d, s]
        vT_sb = qkv_p.tile([128, S], BF16, tag="vT")        # [d, s]
        v_sb = qkv_p.tile([128, NSK, D], BF16, tag="v")     # [s%128, skt, d]
        attnT_sb = qkv_p.tile([128, NH, S], BF16, tag="attnT")

        wq_v = wq_sb.rearrange("p kt n -> p (kt n)")

        # ---------------- phase A: projections + RoPE -----------------
        pha = ExitStack()
        psa_p = pha.enter_context(tc.tile_pool(name="psa", bufs=8, space="PSUM"))
        hid_p = pha.enter_context(tc.tile_pool(name="hid", bufs=2))
        wkv_p = pha.enter_context(tc.tile_pool(name="wkv", bufs=1))
        rope_p = pha.enter_context(tc.tile_pool(name="rope", bufs=3))
        wk_sb = wkv_p.tile([128, KT, D], BF16, tag="wk")
        wv_sb = wkv_p.tile([128, KT, D], BF16, tag="wv")
        for sb in range(NSB):
            ssl = slice(sb * SB, (sb + 1) * SB)
            hid_sb = hid_p.tile([128, KT, SB], BF16, tag="hid")
            hid_v = hid_sb.rearrange("p kt s -> p (kt s)")
            # strip DMA in kt-group chunks (contiguous on both sides);
            # finer chunks on the first strip so the first matmul starts early
            bounds = [0, 2, 4, 8, 16, 24, KT] if sb == 0 else \
                     list(range(0, KT + 1, KG))
            for g0, g1 in zip(bounds[:-1], bounds[1:]):
                nc.sync.dma_start(
                    out=hid_v[:, g0 * SB:g1 * SB],
                    in_=hid_d[:][sb, :, g0 * SB:g1 * SB])
                if sb == 0:
                    gb = NH * D
                    nc.scalar.dma_start(
                        out=wq_v[:, g0 * gb:g1 * gb],
                        in_=wq_d[:][:, g0 * gb:g1 * gb])
            if sb == 0:
                nc.scalar.dma_start(out=wk_sb.rearrange("p kt n -> p (kt n)"),
                                    in_=wk_d[:])
                nc.scalar.dma_start(out=wv_sb.rearrange("p kt n -> p (kt n)"),
                                    in_=wv_d[:])
                nc.scalar.dma_start(out=cos_sb, in_=cos_d[:])
                nc.scalar.dma_start(out=sinA_sb, in_=sinA_d[:])
                nc.scalar.dma_start(out=tri_sb, in_=tri_d[:])
            for t in range(NH + 2):  # 0..3 q heads, 4 = k, 5 = v
                ps = psa_p.tile([128, SB], F32, tag="psa")
                for kt in range(KT):
                    if t < NH:
                        lhsT = wq_sb[:, kt, t * D:(t + 1) * D]
                    elif t == NH:
                        lhsT = wk_sb[:, kt, :]
                    else:
                        lhsT = wv_sb[:, kt, :]
                    nc.tensor.matmul(ps, lhsT, hid_sb[:, kt, :],
                                     start=(kt == 0), stop=(kt == KT - 1))
                if t <= NH:
                    # RoPE: x*cos + rot(x)*sin, rot = [-x2, x1] (partition halves)
                    t1 = rope_p.tile([128, SB], F32, tag="t1")
                    t2 = rope_p.tile([128, SB], F32, tag="t2")
                    nc.vector.tensor_mul(t1, ps, cos_sb[:, ssl])
                    nc.vector.tensor_mul(t2[0:64, :], ps[64:128, :],
                                         sinA_sb[0:64, ssl])
                    nc.vector.tensor_mul(t2[64:128, :], ps[0:64, :],
                                         sinA_sb[64:128, ssl])
                    dst = qT_sb[:, t, ssl] if t < NH else kT_sb[:, ssl]
                    nc.vector.tensor_add(dst, t1, t2)
                else:
                    # v: evacuate to bf16; transposed in one batch at end of
                    # phase A (minimizes DMA xbar-mode transitions)
                    nc.scalar.copy(vT_sb[:, ssl], ps)

        for skt in range(NSK):
            nc.sync.dma_start_transpose(
                out=v_sb[:, skt, :], in_=vT_sb[:, skt * 128:(skt + 1) * 128])
        pha.close()

        # ---------------- phase B: attention per head -----------------
        phb = ExitStack()
        ps_p = ctx.enter_context(tc.tile_pool(name="ps", bufs=5, space="PSUM"))
        psat_p = phb.enter_context(tc.tile_pool(name="psat", bufs=2, space="PSUM"))
        psden_p = phb.enter_context(tc.tile_pool(name="psden", bufs=1, space="PSUM"))
        exp_p = phb.enter_context(tc.tile_pool(name="expp", bufs=12))
        rec_p = phb.enter_context(tc.tile_pool(name="recp", bufs=4))
        for h in range(NH):
            for b in range(NSQB):
                qsl = slice(b * SQB, (b + 1) * SQB)
                nsk = (b + 1) * (SQB // 128)
                ps_at = psat_p.tile([128, SQB], F32, tag="at")
                ps_den = psden_p.tile([1, SQB], F32, tag="den")
                for skt in range(nsk):
                    # diagonal blocks (j >= 0): columns sq < skt*128 are fully
                    # causal-masked - skip them in QK/PV/den and zero them in e
                    j = skt - b * (SQB // 128)
                    lo = max(j, 0) * 128      # first live column in this block
                    ps_sc = ps_p.tile([128, SQB], F32, tag="ps")
                    nc.tensor.matmul(ps_sc[:, lo:],
                                     kT_sb[:, skt * 128:(skt + 1) * 128],
                                     qT_sb[:, h, b * SQB + lo:(b + 1) * SQB],
                                     start=True, stop=True)
                    e = exp_p.tile([128, SQB], BF16, tag="e")
                    nc.scalar.activation(e[:, lo:], ps_sc[:, lo:],
                                         mybir.ActivationFunctionType.Exp,
                                         scale=INV_NORM)
                    if j >= 0:
                        # triangular mask on the [128,128] diagonal tile
                        nc.vector.tensor_mul(e[:, lo:lo + 128],
                                             e[:, lo:lo + 128], tri_sb)
                    nc.tensor.matmul(ps_at[:, lo:], v_sb[:, skt, :], e[:, lo:],
                                     start=(skt == 0), stop=(skt == nsk - 1))
                    nc.tensor.matmul(ps_den[:, lo:], ones_sb, e[:, lo:],
                                     start=(skt == 0), stop=(skt == nsk - 1))
                rec = rec_p.tile([1, SQB], F32, tag="rec")
                nc.vector.reciprocal(rec, ps_den)
                recb = rec_p.tile([128, SQB], F32, tag="recb")
                nc.gpsimd.partition_broadcast(recb, rec)
                nc.vector.tensor_mul(attnT_sb[:, h, qsl], ps_at, recb)

        phb.close()

        # ---------------- phase C: output projection ------------------
        out_p = ctx.enter_context(tc.tile_pool(name="outp", bufs=2))
        wo_sb = wqo_p.tile([128, NH, H], BF16, tag="wqo")
        nc.scalar.dma_start(out=wo_sb.rearrange("p h m -> p (h m)"), in_=wo_d[:])
        NMB = H // SQB  # 8 column blocks of 512
        HB = NMB // 2   # 4 blocks per half-row
        for sqt in range(S // 128):
            for half in range(2):
                o_sb = out_p.tile([128, HB * SQB], F32, tag="o")
                pss = [ps_p.tile([128, SQB], F32, tag="ps",
                                 name=f"pso_{sqt}_{half}_{i}")
                       for i in range(HB)]
                for hh in range(NH):
                    lhsT = attnT_sb[:, hh, sqt * 128:(sqt + 1) * 128]
                    for i in range(HB):
                        mb = half * HB + i
                        nc.tensor.matmul(pss[i], lhsT,
                                         wo_sb[:, hh, mb * SQB:(mb + 1) * SQB],
                                         start=(hh == 0), stop=(hh == NH - 1))
                last = (sqt == S // 128 - 1)
                for i in range(HB):
                    if i % 2 == 0:
                        nc.scalar.copy(o_sb[:, i * SQB:(i + 1) * SQB], pss[i])
                    else:
                        nc.vector.tensor_copy(o_sb[:, i * SQB:(i + 1) * SQB],
                                              pss[i])
                    if last:
                        # fine-grained tail DMAs so the drain isn't gated on
                        # one big final transfer
                        mb = half * HB + i
                        nc.sync.dma_start(
                            out=out_d[:][sqt * 128:(sqt + 1) * 128,
                                         mb * SQB:(mb + 1) * SQB],
                            in_=o_sb[:, i * SQB:(i + 1) * SQB])
                if not last:
                    nc.sync.dma_start(
                        out=out_d[:][sqt * 128:(sqt + 1) * 128,
                                     half * HB * SQB:(half + 1) * HB * SQB],
                        in_=o_sb)

    nc.compile()
    return nc


_NC = None


def _get_nc():
    global _NC
    if _NC is None:
        _NC = _build_program()
    return _NC


def _host_tables():
    pos = np.arange(S, dtype=np.float32)
    inv_freq = 1.0 / (MAX_WAVELENGTH ** (np.arange(0, D, 2, dtype=np.float32) / D))
    freq = np.einsum('i,j->ij', pos, inv_freq)          # [S, 64]
    emb = np.concatenate([freq, freq], axis=1)          # [S, 128]
    cosT = np.ascontiguousarray(np.cos(emb).T).astype(np.float32)   # [128, S]
    sinT = np.sin(emb).T.astype(np.float32)
    sinA = sinT.copy()
    sinA[:64] = -sinT[:64]
    sinA = np.ascontiguousarray(sinA)
    tri = np.triu(np.ones((D, D), dtype=np.float32)).astype(NPBF16)  # p<=f keep
    return cosT, sinA, tri


def _prepare_in_maps(hidden_states, wq, wk, wv, wo):
    hs = np.asarray(hidden_states, dtype=np.float32)[0]        # [S, H]
    wq = np.asarray(wq, dtype=np.float32)                      # [H, 32, 128]
    wk = np.asarray(wk, dtype=np.float32)                      # [H, 8, 128]
    wv = np.asarray(wv, dtype=np.float32)
    wo = np.asarray(wo, dtype=np.float32)                      # [32, 128, H]

    # hidP[sb, p, kt*SB + s] = hiddenT[kt*128 + p, sb*SB + s]
    hidT = hs.T.astype(NPBF16)                                 # [H, S]
    hidP = np.ascontiguousarray(
        hidT.reshape(KT, 128, NSB, SB).transpose(2, 1, 0, 3).reshape(
            NSB, 128, KT * SB))
    cosT, sinA, tri = _host_tables()

    def ptile(w2d):  # [H, N] -> [128, KT*N] with (p, kt*N+n) = w2d[kt*128+p, n]
        n = w2d.shape[1]
        return np.ascontiguousarray(
            w2d.reshape(KT, 128, n).transpose(1, 0, 2).reshape(128, KT * n))

    in_maps = []
    for c in range(NCORES):
        wq_c = wq[:, NH * c:NH * (c + 1), :].reshape(H, NH * D).astype(NPBF16)
        wk_c = wk[:, c, :].astype(NPBF16)
        wv_c = wv[:, c, :].astype(NPBF16)
        wo_c = wo[NH * c:NH * (c + 1)].reshape(NH * D, H).astype(NPBF16)
        woP = np.ascontiguousarray(
            wo_c.reshape(NH, 128, H).transpose(1, 0, 2).reshape(128, NH * H))
        in_maps.append({
            "hidP": hidP,
            "wqP": ptile(wq_c),
            "wkP": ptile(wk_c),
            "wvP": ptile(wv_c),
            "woP": woP,
            "cosT": cosT,
            "sinA": sinA,
            "trimask": tri,
        })
    return in_maps


def _run(in_maps, **kwargs):
    return run_bass_kernel_spmd(_get_nc(), in_maps,
                                core_ids=list(range(NCORES)), **kwargs)


def _gather(res):
    out = np.zeros((S, H), dtype=np.float32)
    for c in range(NCORES):
        out += np.asarray(res.results[c]["out"], dtype=np.float32)
    return out[None]


def kernel(hidden_states, attention_mask=None, wq=None, wk=None, wv=None, wo=None):
    in_maps = _prepare_in_maps(hidden_states, wq, wk, wv, wo)
    return _gather(_run(in_maps))


# revision 21
# speedup vs baseline: 1.0083x; 1.0059x over previous
"""Trainium2 Bass kernel for CachedMistralAttention prefill (B=1, S=2048, H=4096,
32 q heads / 8 kv heads GQA, rotate-half RoPE, causal SDPA).

Sharding: tensor-parallel over heads across 8 NeuronCores. Core c owns q heads
[4c, 4c+4) and kv head c (one GQA group), computes its partial output
projection attn @ wo[4c:4c+4], and the host sums the 8 partials.

Per-core dataflow (all matmuls bf16 with fp32 PSUM accumulation):
  phase A: qT/kT/vT = W.T @ hiddenT per s-block; RoPE applied on the fly
           (rotate-half via partition-offset DVE reads); v transposed to
           [s, d] via DMA-transpose for use as the PV stationary operand.
  phase B: per (head, 512-wide sq block): scoresT[sk, sq] = kT.T @ qT tiles,
           exp on ScalarE (scale=1/sqrt(d)) with causal masking (skip upper
           tiles, zero left strip + triangular mask on diagonal tiles),
           attn_unnormT[d, sq] = v.T @ expT and denom[1, sq] = ones.T @ expT
           accumulated on PSUM, reciprocal + partition_broadcast + DVE mult
           for the normalization.
  phase C: out[sq, m] = sum_h attnT_h.T @ wo_h accumulated in PSUM over
           heads, evacuated to SBUF and DMA'd out as fp32.

All DRAM inputs are pre-tiled on the host so each DMA reads per-partition
contiguous spans (few, large descriptors - DMA issue is the scarce resource).
"""

import math
from contextlib import ExitStack

import numpy as np
import ml_dtypes

import concourse.bacc as bacc
import concourse.mybir as mybir
import concourse.tile as tile
from concourse.bass_utils import run_bass_kernel_spmd

BF16 = mybir.dt.bfloat16
F32 = mybir.dt.float32
NPBF16 = ml_dtypes.bfloat16

S = 2048          # sequence length
H = 4096          # hidden size
D = 128           # head dim
NH = 4            # q heads per core (one GQA group)
NCORES = 8
KT = H // 128     # 32 contraction tiles for the projections
SB = 512          # phase A s-block width
NSB = S // SB     # 4
KG = 16           # kt-tiles per hidden-strip sub-DMA chunk
SQB = 512         # phase B sq-block width
NSQB = S // SQB   # 4
NSK = S // 128    # 16 sk tiles
INV_NORM = 1.0 / math.sqrt(D)
MAX_WAVELENGTH = 10000.0


def _build_program():
    nc = bacc.Bacc("TRN2", target_bir_lowering=False, debug=False,
                   num_devices=NCORES)

    # pre-tiled inputs: leading dim 128 = SBUF partition, free dims contiguous
    hid_d = nc.dram_tensor("hidP", [NSB, 128, KT * SB], BF16, kind="ExternalInput")
    wq_d = nc.dram_tensor("wqP", [128, KT * NH * D], BF16, kind="ExternalInput")
    wk_d = nc.dram_tensor("wkP", [128, KT * D], BF16, kind="ExternalInput")
    wv_d = nc.dram_tensor("wvP", [128, KT * D], BF16, kind="ExternalInput")
    wo_d = nc.dram_tensor("woP", [128, NH * H], BF16, kind="ExternalInput")
    cos_d = nc.dram_tensor("cosT", [D, S], F32, kind="ExternalInput")
    sinA_d = nc.dram_tensor("sinA", [D, S], F32, kind="ExternalInput")
    tri_d = nc.dram_tensor("trimask", [D, D], BF16, kind="ExternalInput")
    out_d = nc.dram_tensor("out", [S, H], F32, kind="ExternalOutput")

    with tile.TileContext(nc) as tc, ExitStack() as ctx:
        # pools
        wqo_p = ctx.enter_context(tc.tile_pool(name="wqo", bufs=1))
        const_p = ctx.enter_context(tc.tile_pool(name="const", bufs=1))
        qkv_p = ctx.enter_context(tc.tile_pool(name="qkv", bufs=1))
        ps_p = None  # created per phase

        wq_sb = wqo_p.tile([128, KT, NH * D], BF16, tag="wqo")
        cos_sb = const_p.tile([D, S], F32, tag="cos")
        sinA_sb = const_p.tile([D, S], F32, tag="sin")
        tri_sb = const_p.tile([D, D], BF16, tag="tri")
        ones_sb = const_p.tile([128, 1], BF16, tag="ones")
        nc.vector.memset(ones_sb, 1.0)

        # persistent activations
        qT_sb = qkv_p.tile([128, NH, S], BF16, tag="qT")    # [d, h, s]
        kT_sb = qkv_p.tile([128, S], BF16, tag="kT")        # [d, s]
        vT_sb = qkv_p.tile([128, S], BF16, tag="vT")        # [d, s]
        v_sb = qkv_p.tile([128, NSK, D], BF16, tag="v")     # [s%128, skt, d]
        attnT_sb = qkv_p.tile([128, NH, S], BF16, tag="attnT")

        wq_v = wq_sb.rearrange("p kt n -> p (kt n)")

        # ---------------- phase A: projections + RoPE -----------------
        pha = ExitStack()
        psa_p = pha.enter_context(tc.tile_pool(name="psa", bufs=8, space="PSUM"))
        hid_p = pha.enter_context(tc.tile_pool(name="hid", bufs=2))
        wkv_p = pha.enter_context(tc.tile_pool(name="wkv", bufs=1))
        rope_p = pha.enter_context(tc.tile_pool(name="rope", bufs=3))
        wk_sb = wkv_p.tile([128, KT, D], BF16, tag="wk")
        wv_sb = wkv_p.tile([128, KT, D], BF16, tag="wv")
        for sb in range(NSB):
            ssl = slice(sb * SB, (sb + 1) * SB)
            hid_sb = hid_p.tile([128, KT, SB], BF16, tag="hid")
            hid_v = hid_sb.rearrange("p kt s -> p (kt s)")
            # strip DMA in kt-group chunks (contiguous on both sides);
            # finer chunks on the first strip so the first matmul starts early
            bounds = [0, 2, 4, 8, 16, 24, KT] if sb == 0 else \
                     list(range(0, KT + 1, KG))
            for g0, g1 in zip(bounds[:-1], bounds[1:]):
                nc.sync.dma_start(
                    out=hid_v[:, g0 * SB:g1 * SB],
                    in_=hid_d[:][sb, :, g0 * SB:g1 * SB])
                if sb == 0:
                    gb = NH * D
                    nc.scalar.dma_start(
                        out=wq_v[:, g0 * gb:g1 * gb],
                        in_=wq_d[:][:, g0 * gb:g1 * gb])
            if sb == 0:
                nc.scalar.dma_start(out=wk_sb.rearrange("p kt n -> p (kt n)"),
                                    in_=wk_d[:])
                nc.scalar.dma_start(out=wv_sb.rearrange("p kt n -> p (kt n)"),
                                    in_=wv_d[:])
                nc.scalar.dma_start(out=cos_sb, in_=cos_d[:])
                nc.scalar.dma_start(out=sinA_sb, in_=sinA_d[:])
                nc.scalar.dma_start(out=tri_sb, in_=tri_d[:])
            for t in range(NH + 2):  # 0..3 q heads, 4 = k, 5 = v
                ps = psa_p.tile([128, SB], F32, tag="psa")
                for kt in range(KT):
                    if t < NH:
                        lhsT = wq_sb[:, kt, t * D:(t + 1) * D]
                    elif t == NH:
                        lhsT = wk_sb[:, kt, :]
                    else:
                        lhsT = wv_sb[:, kt, :]
                    nc.tensor.matmul(ps, lhsT, hid_sb[:, kt, :],
                                     start=(kt == 0), stop=(kt == KT - 1))
                if t <= NH:
                    # RoPE: x*cos + rot(x)*sin, rot = [-x2, x1] (partition halves)
                    t1 = rope_p.tile([128, SB], F32, tag="t1")
                    t2 = rope_p.tile([128, SB], F32, tag="t2")
                    nc.vector.tensor_mul(t1, ps, cos_sb[:, ssl])
                    nc.vector.tensor_mul(t2[0:64, :], ps[64:128, :],
                                         sinA_sb[0:64, ssl])
                    nc.vector.tensor_mul(t2[64:128, :], ps[0:64, :],
                                         sinA_sb[64:128, ssl])
                    dst = qT_sb[:, t, ssl] if t < NH else kT_sb[:, ssl]
                    nc.vector.tensor_add(dst, t1, t2)
                else:
                    # v: evacuate to bf16; transposed in one batch at end of
                    # phase A (minimizes DMA xbar-mode transitions)
                    nc.scalar.copy(vT_sb[:, ssl], ps)

        for skt in range(NSK):
            nc.sync.dma_start_transpose(
                out=v_sb[:, skt, :], in_=vT_sb[:, skt * 128:(skt + 1) * 128])
        pha.close()

        # ---------------- phase B: attention per head -----------------
        phb = ExitStack()
        ps_p = ctx.enter_context(tc.tile_pool(name="ps", bufs=5, space="PSUM"))
        psat_p = phb.enter_context(tc.tile_pool(name="psat", bufs=2, space="PSUM"))
        psden_p = phb.enter_context(tc.tile_pool(name="psden", bufs=1, space="PSUM"))
        exp_p = phb.enter_context(tc.tile_pool(name="expp", bufs=12))
        rec_p = phb.enter_context(tc.tile_pool(name="recp", bufs=4))
        for b in range(NSQB):
            for h in range(NH):
                qsl = slice(b * SQB, (b + 1) * SQB)
                nsk = (b + 1) * (SQB // 128)
                ps_at = psat_p.tile([128, SQB], F32, tag="at")
                ps_den = psden_p.tile([1, SQB], F32, tag="den")
                for skt in range(nsk):
                    # diagonal blocks (j >= 0): columns sq < skt*128 are fully
                    # causal-masked - skip them in QK/PV/den and zero them in e
                    j = skt - b * (SQB // 128)
                    lo = max(j, 0) * 128      # first live column in this block
                    ps_sc = ps_p.tile([128, SQB], F32, tag="ps")
                    nc.tensor.matmul(ps_sc[:, lo:],
                                     kT_sb[:, skt * 128:(skt + 1) * 128],
                                     qT_sb[:, h, b * SQB + lo:(b + 1) * SQB],
                                     start=True, stop=True)
                    e = exp_p.tile([128, SQB], BF16, tag="e")
                    nc.scalar.activation(e[:, lo:], ps_sc[:, lo:],
                                         mybir.ActivationFunctionType.Exp,
                                         scale=INV_NORM)
                    if j >= 0:
                        # triangular mask on the [128,128] diagonal tile
                        nc.vector.tensor_mul(e[:, lo:lo + 128],
                                             e[:, lo:lo + 128], tri_sb)
                    nc.tensor.matmul(ps_at[:, lo:], v_sb[:, skt, :], e[:, lo:],
                                     start=(skt == 0), stop=(skt == nsk - 1))
                    nc.tensor.matmul(ps_den[:, lo:], ones_sb, e[:, lo:],
                                     start=(skt == 0), stop=(skt == nsk - 1))
                rec = rec_p.tile([1, SQB], F32, tag="rec")
                nc.vector.reciprocal(rec, ps_den)
                recb = rec_p.tile([128, SQB], F32, tag="recb")
                nc.gpsimd.partition_broadcast(recb, rec)
                nc.vector.tensor_mul(attnT_sb[:, h, qsl], ps_at, recb)

        phb.close()

        # ---------------- phase C: output projection ------------------
        out_p = ctx.enter_context(tc.tile_pool(name="outp", bufs=2))
        wo_sb = wqo_p.tile([128, NH, H], BF16, tag="wqo")
        nc.scalar.dma_start(out=wo_sb.rearrange("p h m -> p (h m)"), in_=wo_d[:])
        NMB = H // SQB  # 8 column blocks of 512
        HB = NMB // 2   # 4 blocks per half-row
        for sqt in range(S // 128):
            for half in range(2):
                o_sb = out_p.tile([128, HB * SQB], F32, tag="o")
                pss = [ps_p.tile([128, SQB], F32, tag="ps",
                                 name=f"pso_{sqt}_{half}_{i}")
                       for i in range(HB)]
                for hh in range(NH):
                    lhsT = attnT_sb[:, hh, sqt * 128:(sqt + 1) * 128]
                    for i in range(HB):
                        mb = half * HB + i
                        nc.tensor.matmul(pss[i], lhsT,
                                         wo_sb[:, hh, mb * SQB:(mb + 1) * SQB],
                                         start=(hh == 0), stop=(hh == NH - 1))
                last = (sqt == S // 128 - 1)
                for i in range(HB):
                    if i % 2 == 0:
                        nc.scalar.copy(o_sb[:, i * SQB:(i + 1) * SQB], pss[i])
                    else:
                        nc.vector.tensor_copy(o_sb[:, i * SQB:(i + 1) * SQB],
                                              pss[i])
                    if last:
                        # fine-grained tail DMAs so the drain isn't gated on
                        # one big final transfer
                        mb = half * HB + i
                        nc.sync.dma_start(
                            out=out_d[:][sqt * 128:(sqt + 1) * 128,
                                         mb * SQB:(mb + 1) * SQB],
                            in_=o_sb[:, i * SQB:(i + 1) * SQB])
                if not last:
                    nc.sync.dma_start(
                        out=out_d[:][sqt * 128:(sqt + 1) * 128,
                                     half * HB * SQB:(half + 1) * HB * SQB],
                        in_=o_sb)

    nc.compile()
    return nc


_NC = None


def _get_nc():
    global _NC
    if _NC is None:
        _NC = _build_program()
    return _NC


def _host_tables():
    pos = np.arange(S, dtype=np.float32)
    inv_freq = 1.0 / (MAX_WAVELENGTH ** (np.arange(0, D, 2, dtype=np.float32) / D))
    freq = np.einsum('i,j->ij', pos, inv_freq)          # [S, 64]
    emb = np.concatenate([freq, freq], axis=1)          # [S, 128]
    cosT = np.ascontiguousarray(np.cos(emb).T).astype(np.float32)   # [128, S]
    sinT = np.sin(emb).T.astype(np.float32)
    sinA = sinT.copy()
    sinA[:64] = -sinT[:64]
    sinA = np.ascontiguousarray(sinA)
    tri = np.triu(np.ones((D, D), dtype=np.float32)).astype(NPBF16)  # p<=f keep
    return cosT, sinA, tri


def _prepare_in_maps(hidden_states, wq, wk, wv, wo):
    hs = np.asarray(hidden_states, dtype=np.float32)[0]        # [S, H]
    wq = np.asarray(wq, dtype=np.float32)                      # [H, 32, 128]
    wk = np.asarray(wk, dtype=np.float32)                      # [H, 8, 128]
    wv = np.asarray(wv, dtype=np.float32)
    wo = np.asarray(wo, dtype=np.float32)                      # [32, 128, H]

    # hidP[sb, p, kt*SB + s] = hiddenT[kt*128 + p, sb*SB + s]
    hidT = hs.T.astype(NPBF16)                                 # [H, S]
    hidP = np.ascontiguousarray(
        hidT.reshape(KT, 128, NSB, SB).transpose(2, 1, 0, 3).reshape(
            NSB, 128, KT * SB))
    cosT, sinA, tri = _host_tables()

    def ptile(w2d):  # [H, N] -> [128, KT*N] with (p, kt*N+n) = w2d[kt*128+p, n]
        n = w2d.shape[1]
        return np.ascontiguousarray(
            w2d.reshape(KT, 128, n).transpose(1, 0, 2).reshape(128, KT * n))

    in_maps = []
    for c in range(NCORES):
        wq_c = wq[:, NH * c:NH * (c + 1), :].reshape(H, NH * D).astype(NPBF16)
        wk_c = wk[:, c, :].astype(NPBF16)
        wv_c = wv[:, c, :].astype(NPBF16)
        wo_c = wo[NH * c:NH * (c + 1)].reshape(NH * D, H).astype(NPBF16)
        woP = np.ascontiguousarray(
            wo_c.reshape(NH, 128, H).transpose(1, 0, 2).reshape(128, NH * H))
        in_maps.append({
            "hidP": hidP,
            "wqP": ptile(wq_c),
            "wkP": ptile(wk_c),
            "wvP": ptile(wv_c),
            "woP": woP,
            "cosT": cosT,
            "sinA": sinA,
            "trimask": tri,
        })
    return in_maps


def _run(in_maps, **kwargs):
    return run_bass_kernel_spmd(_get_nc(), in_maps,
                                core_ids=list(range(NCORES)), **kwargs)


def _gather(res):
    out = np.zeros((S, H), dtype=np.float32)
    for c in range(NCORES):
        out += np.asarray(res.results[c]["out"], dtype=np.float32)
    return out[None]


def kernel(hidden_states, attention_mask=None, wq=None, wk=None, wv=None, wo=None):
    in_maps = _prepare_in_maps(hidden_states, wq, wk, wv, wo)
    return _gather(_run(in_maps))
